# revision 2
# baseline (speedup 1.0000x reference)
"""TRN2 Bass kernel for the EvolvedNet GNN message-passing problem.

Problem: 64 relaxation passes over a 64-node graph with 256 weighted edges;
each pass sequentially applies  vals[dst_e] += tanh(w_e * vals[src_e])  for
e = 0..255; batch of 8192 independent columns; output = tanh(vals[16:24]).

Strategy
--------
Batch is sharded over 8 NeuronCores (1024 columns each).  Per core the batch
is laid out batch-on-partitions: 128 partitions x 8 subtile elements, with
node n occupying 8 contiguous free-axis elements at column 8*perm[n] of a
[128, 512] SBUF state tile.

The 256-edge tape of one pass is level-scheduled: level(e) honors exact
RAW/WAR/WAW ordering, giving ~13 dependency levels per pass.  Per level:
  DVE: tmp[slot(e)] = v[src_e] * w_tape[slot(e)]  (packed multi-edge APs)
  ACT: t[slots]     = tanh(tmp[slots])            (one batched instruction)
  DVE: v[dst_e]    += t[slot(e)]                  (packed; same-dst adds in
                                                   tape order => fp-exact)
Edges are packed into single instructions via strided access patterns:
arithmetic runs of any length, 2x2 affine grids ("quads"), and pairs, found
by a greedy miner over a tuned node-column permutation.  Leftover edges use
a direct ACT tanh-with-scale.  TileContext supplies all semaphores from the
byte-exact dependency trace; the schedule is fp-bit-exact vs. the reference
scan (modulo the hardware tanh table).
"""

import dataclasses
import os
from collections import defaultdict

import numpy as np

import concourse.bass as bass
import concourse.mybir as mybir
from concourse.tile import TileContext
from concourse.bass_utils import run_bass_kernel_spmd

NODES = 64
NPART = 128
SUB = 8
N_CORES = 8
N_PASSES = 64
BATCH = 8192
N_IN = 16
N_OUT = 8
E = 256

AP = bass.AP
F32 = mybir.dt.float32
Tanh = mybir.ActivationFunctionType.Tanh
MULT = mybir.AluOpType.mult
ADD = mybir.AluOpType.add

# Node-column permutation tuned (simulated annealing) for the deterministic
# problem graph; any permutation is semantically valid for any graph.
BEST_PERM = [59, 23, 42, 36, 37, 52, 31, 18, 27, 4, 26, 28, 11, 25, 32, 43, 49, 30, 13, 22, 15, 8, 57, 51, 3, 58, 1, 39, 38, 41, 14, 40, 54, 55, 6, 19, 0, 5, 33, 24, 21, 9, 16, 12, 53, 10, 61, 45, 56, 50, 63, 62, 29, 20, 60, 34, 47, 44, 17, 35, 2, 48, 7, 46]


# ---------------- schedule ----------------

def compute_levels(src, dst):
    L = np.zeros(len(src), np.int32)
    last_w, last_r = {}, {}
    for i in range(len(src)):
        s, d = int(src[i]), int(dst[i])
        lv = 1
        if s in last_w:
            lv = max(lv, last_w[s] + 1)
        if d in last_r:
            lv = max(lv, last_r[d])
        if d in last_w:
            lv = max(lv, last_w[d])
        L[i] = lv
        last_w[d] = max(last_w.get(d, 0), lv)
        last_r[s] = max(last_r.get(s, 0), lv)
    return L


def mine_runs(items, colfn, min_len=3):
    pool = list(items)
    runs = []
    while len(pool) >= min_len:
        bycol = defaultdict(list)
        for e in pool:
            bycol[colfn(e)].append(e)
        cols = list(bycol.keys())
        best = None
        tried = set()
        for ci in cols:
            for cj in cols:
                if ci == cj:
                    continue
                d = tuple(b - a for a, b in zip(ci, cj))
                if (ci, d) in tried:
                    continue
                tried.add((ci, d))
                run_cols = [ci]
                cur = cj
                while cur in bycol:
                    run_cols.append(cur)
                    cur = tuple(a + b for a, b in zip(cur, d))
                if len(run_cols) >= min_len and (
                        best is None or len(run_cols) > len(best)):
                    best = run_cols
        if best is None:
            break
        run = [bycol[c].pop() for c in best]
        runs.append(run)
        cnt = defaultdict(int)
        for e in run:
            cnt[e] += 1
        newpool = []
        for e in pool:
            if cnt.get(e, 0) > 0:
                cnt[e] -= 1
            else:
                newpool.append(e)
        pool = newpool
    return runs, pool


def quads_and_pairs(edges, keyfn):
    es = list(edges)
    used = set()
    packs = []
    buckets = defaultdict(list)
    n = len(es)
    for i in range(n):
        for j in range(i + 1, n):
            buckets[keyfn(es[i], es[j])].append((es[i], es[j]))
    for delta, pairs in sorted(buckets.items(), key=lambda kv: -len(kv[1])):
        if len(pairs) < 2:
            continue
        taken, busy = [], set()
        for p in pairs:
            if p[0] in used or p[1] in used or p[0] in busy or p[1] in busy:
                continue
            taken.append(p)
            busy.add(p[0])
            busy.add(p[1])
        while len(taken) >= 2:
            p1 = taken.pop(0)
            p2 = taken.pop(0)
            packs.append([p1[0], p1[1], p2[0], p2[1]])
            used.update(packs[-1])
    rest = [e for e in es if e not in used]
    while len(rest) >= 2:
        packs.append([rest.pop(0), rest.pop(0)])
    return packs, rest


def build_schedule(src, dst, perm):
    L = compute_levels(src, dst)
    n_levels = int(L.max())
    levels = []
    slot_counter = 0
    for lv in range(1, n_levels + 1):
        edges = sorted(i for i in range(E) if L[i] == lv)
        rank = defaultdict(int)
        waves = defaultdict(list)
        for e in edges:
            waves[rank[int(dst[e])]].append(e)
            rank[int(dst[e])] += 1
        w0 = waves[0]
        others = [e for wv in range(1, len(waves)) for e in waves[wv]]

        def scol_of(e):
            return perm[src[e]]

        def dcol_of(e):
            return perm[dst[e]]

        jruns, w0_rest = mine_runs(w0, lambda e: (scol_of(e), dcol_of(e)))
        jqp, _ = quads_and_pairs(
            w0_rest, lambda a, b: (scol_of(b) - scol_of(a),
                                   dcol_of(b) - dcol_of(a)))
        joint_packs = jruns + jqp
        sruns, rest = mine_runs(others, lambda e: (scol_of(e),))
        sqp, _ = quads_and_pairs(
            rest, lambda a, b: (scol_of(b) - scol_of(a),))
        src_packs = sruns + sqp
        packed = set()
        for p in joint_packs + src_packs:
            packed.update(p)
        act_singles = [e for e in edges if e not in packed]
        mult_packs = joint_packs + src_packs

        slot = {}
        lo = slot_counter
        for p in mult_packs:
            for e in p:
                slot[e] = slot_counter
                slot_counter += 1
        hi = slot_counter
        for e in act_singles:
            slot[e] = slot_counter
            slot_counter += 1

        add_waves = []
        joint_set = set()
        for p in joint_packs:
            joint_set.update(p)
        for wv in sorted(waves.keys()):
            wedges = [e for e in waves[wv] if e not in joint_set]
            aruns, arest = mine_runs(wedges, lambda e: (dcol_of(e), slot[e]))
            aqp, asingles = quads_and_pairs(
                arest, lambda a, b: (dcol_of(b) - dcol_of(a),
                                     slot[b] - slot[a]))
            packs = aruns + aqp
            if wv == 0:
                packs = joint_packs + packs
            add_waves.append(dict(packs=packs, singles=asingles))
        levels.append(dict(lv=lv, edges=edges, mult_packs=mult_packs,
                           act_singles=act_singles, add_waves=add_waves,
                           slot=slot, batch_range=(lo, hi)))
    assert slot_counter <= E
    return levels


# ---------------- access patterns ----------------

def grid_ap(tensor, pack_cols, free_width, part_stride):
    k = len(pack_cols)
    if k == 1:
        return AP(tensor, pack_cols[0],
                  [[part_stride, NPART], [1, free_width]])
    d = pack_cols[1] - pack_cols[0]
    if all(pack_cols[i + 1] - pack_cols[i] == d for i in range(k - 1)):
        return AP(tensor, pack_cols[0],
                  [[part_stride, NPART], [d, k], [1, free_width]])
    if k == 4:
        d_in = pack_cols[1] - pack_cols[0]
        assert pack_cols[3] - pack_cols[2] == d_in, pack_cols
        return AP(tensor, pack_cols[0],
                  [[part_stride, NPART],
                   [pack_cols[2] - pack_cols[0], 2],
                   [d_in, 2], [1, free_width]])
    raise ValueError(pack_cols)


# ---------------- walrus workaround ----------------

def split_waits(nc, max_waits=1):
    """This walrus build rejects >1 sync-wait per instruction; split extra
    waits onto chained NoOps preceding the instruction."""
    for bb in nc.main_func.blocks:
        new_instrs = []
        for ins in bb.instructions:
            si = getattr(ins, "sync_info", None)
            if si is not None and si.on_wait and len(si.on_wait) > max_waits:
                waits = list(si.on_wait)
                k = 0
                while len(waits) > max_waits:
                    chunk, waits = waits[:max_waits], waits[max_waits:]
                    nop = mybir.InstNoOp(name=f"{ins.name}_ws{k}")
                    nop.engine = ins.engine
                    nop.sync_info = mybir.SyncInfo(on_wait=chunk, on_update=[])
                    new_instrs.append(nop)
                    k += 1
                ins.sync_info = dataclasses.replace(si, on_wait=waits)
            new_instrs.append(ins)
        bb.instructions = new_instrs
    return nc


# ---------------- kernel builder ----------------

def build_kernel(src, dst, w, n_passes, perm, split=True):
    src = [int(x) for x in src]
    dst = [int(x) for x in dst]
    w = np.asarray(w, np.float32)
    levels = build_schedule(src, dst, perm)

    wtape_np = np.zeros(E, np.float32)
    for lvd in levels:
        for e, s in lvd["slot"].items():
            wtape_np[s] = w[e]

    VW = NODES * SUB
    TW = E * SUB

    nc = bass.Bass()
    xin = nc.dram_tensor("xin", [NPART, VW], F32, kind="ExternalInput")
    wt = nc.dram_tensor("wt", [NPART, TW], F32, kind="ExternalInput")
    yout = nc.dram_tensor("yout", [NPART, N_OUT * SUB], F32,
                          kind="ExternalOutput")

    with TileContext(nc) as tc:
        with tc.tile_pool(name="p", bufs=1) as pool:
            v = pool.tile([NPART, VW], F32)
            tmp = pool.tile([NPART, TW], F32)
            tbuf = pool.tile([NPART, TW], F32)
            wtape = pool.tile([NPART, TW], F32)
            osb = pool.tile([NPART, N_OUT * SUB], F32)

            def vcol(n):
                return perm[n] * SUB

            def scol(s):
                return s * SUB

            nc.sync.dma_start(v[:, :], xin[:, :])
            nc.sync.dma_start(wtape[:, :], wt[:, :])

            vt = v.tensor
            tmpt = tmp.tensor
            tbuft = tbuf.tensor
            wtapet = wtape.tensor

            def emit_mult_pack(p, slot):
                cols = [vcol(src[e]) for e in p]
                slots = [scol(slot[e]) for e in p]
                in_ap = grid_ap(vt, cols, SUB, VW)
                out_ap = grid_ap(tmpt, slots, SUB, TW)
                w_ap = grid_ap(wtapet, slots, SUB, TW)
                nc.vector.tensor_tensor(out_ap, in_ap, w_ap, MULT)

            def emit_add_pack(p, slot):
                cols = [vcol(dst[e]) for e in p]
                slots = [scol(slot[e]) for e in p]
                dst_ap = grid_ap(vt, cols, SUB, VW)
                t_ap = grid_ap(tbuft, slots, SUB, TW)
                nc.vector.tensor_tensor(dst_ap, dst_ap, t_ap, ADD)

            for p_i in range(n_passes):
                for lvd in levels:
                    slot = lvd["slot"]
                    for p in lvd["mult_packs"]:
                        emit_mult_pack(p, slot)
                    lo, hi = lvd["batch_range"]
                    if hi > lo:
                        nc.scalar.activation(tbuf[:, scol(lo):scol(hi)],
                                             tmp[:, scol(lo):scol(hi)], Tanh)
                    for e in lvd["act_singles"]:
                        s = lvd["slot"][e]
                        nc.scalar.activation(
                            tbuf[:, scol(s):scol(s) + SUB],
                            v[:, vcol(src[e]):vcol(src[e]) + SUB],
                            Tanh, scale=float(w[e]))
                    for wave in lvd["add_waves"]:
                        for p in wave["packs"]:
                            emit_add_pack(p, slot)
                        for e in wave["singles"]:
                            emit_add_pack([e], slot)

            for m in range(N_OUT):
                oc = vcol(N_IN + m)
                nc.scalar.activation(osb[:, m * SUB:(m + 1) * SUB],
                                     v[:, oc:oc + SUB], Tanh)
            nc.sync.dma_start(yout[:, :], osb[:, :])

    if split:
        split_waits(nc)
    return nc, wtape_np


# ---------------- host marshalling ----------------

def prep_x_core(x_core, perm):
    """x_core [16, 1024] -> full initial v [128, 512]:
    v[p, 8*perm[n]+k] = x[n, 128k+p] for n<16, zeros elsewhere."""
    out = np.zeros((NPART, NODES, SUB), np.float32)
    xc = np.asarray(x_core, np.float32).reshape(N_IN, SUB, NPART)
    xc = xc.transpose(2, 0, 1)  # [p, n, k]
    for n in range(N_IN):
        out[:, perm[n], :] = xc[:, n, :]
    return np.ascontiguousarray(out.reshape(NPART, NODES * SUB))


def unprep_y_core(y_core):
    """y_core [128, 64] with [p, 8m+k] -> [8, 1024] out[m, 128k+p]."""
    yc = np.asarray(y_core).reshape(NPART, N_OUT, SUB)
    return np.ascontiguousarray(
        yc.transpose(1, 2, 0).reshape(N_OUT, SUB * NPART))


def make_wtape_input(wtape_np):
    rep = np.repeat(np.asarray(wtape_np, np.float32), SUB)[None, :]
    return np.ascontiguousarray(np.broadcast_to(rep, (NPART, rep.shape[1])))


# ---------------- public entry ----------------

def build_for_timing(inputs, n_passes):
    x, w, src, dst = inputs["x"], inputs["w"], inputs["src"], inputs["dst"]
    perm = list(BEST_PERM)
    nc, wtape_np = build_kernel([int(s) for s in src], [int(d) for d in dst],
                                w, n_passes, perm)
    wt_in = make_wtape_input(wtape_np)
    B_core = BATCH // N_CORES
    in_maps = [{"xin": prep_x_core(x[:, c * B_core:(c + 1) * B_core], perm),
                "wt": wt_in} for c in range(N_CORES)]
    return nc, in_maps


_CACHE = {}


def _get_built(src_key, dst_key, w_key, n_passes):
    key = (src_key, dst_key, w_key, n_passes)
    if key not in _CACHE:
        src = np.frombuffer(src_key, np.int32)
        dst = np.frombuffer(dst_key, np.int32)
        w = np.frombuffer(w_key, np.float32)
        perm = list(BEST_PERM)
        nc, wtape_np = build_kernel(src, dst, w, n_passes, perm)
        _CACHE[key] = (nc, wtape_np, perm)
    return _CACHE[key]


def kernel(x, w, src, dst, n_passes=N_PASSES, _n_reps=1):
    """Full-input entry: x [16, 8192] f32, w [256] f32, src/dst [256] i32.
    Returns [8, 8192] f32."""
    x = np.ascontiguousarray(np.asarray(x, np.float32))
    w = np.ascontiguousarray(np.asarray(w, np.float32))
    src = np.ascontiguousarray(np.asarray(src, np.int32))
    dst = np.ascontiguousarray(np.asarray(dst, np.int32))
    assert x.shape == (N_IN, BATCH)

    nc, wtape_np, perm = _get_built(src.tobytes(), dst.tobytes(),
                                    w.tobytes(), n_passes)
    wt_in = make_wtape_input(wtape_np)
    in_maps = []
    B_core = BATCH // N_CORES
    for c in range(N_CORES):
        xc = x[:, c * B_core:(c + 1) * B_core]
        in_maps.append({"xin": prep_x_core(xc, perm), "wt": wt_in})

    # the axon client in this container lacks the NTFF trace hook; make sure
    # an inherited BASS_TRACE=1 cannot route us into that path
    os.environ.setdefault("BASS_NEVER_TRACE", "1")

    results = None
    for _ in range(max(1, _n_reps) + 1):
        # first run warms the ACT table load (known first-exec race);
        # result of the final run is used.
        results = run_bass_kernel_spmd(
            nc, in_maps, core_ids=list(range(N_CORES))).results

    y = np.empty((N_OUT, BATCH), np.float32)
    for c in range(N_CORES):
        y[:, c * B_core:(c + 1) * B_core] = unprep_y_core(results[c]["yout"])
    return y

